# revision 5
# baseline (speedup 1.0000x reference)
"""Fused Fourier-block kernel for TRN2 (8 NeuronCores, data-parallel).

Reference computation (per token, C=1024, H=4096):
    h  = LN1(x)
    f  = real(FFT_C(h)) = h @ COS            (COS[n,k] = cos(2*pi*n*k/C))
    x2 = x + LNf(f)
    h2 = LN2(x2)
    m  = gelu_exact(h2 @ w1 + b1)
    out = x2 + m @ w2 + b2

Strategy: shard the 4*2048 = 8192 tokens over 8 cores (1024 tokens each).
All device math is done with activations CHANNEL-MAJOR ([channel, token]),
so every matmul consumes weights in their natural [in, out] layout and
chains without any device-side transposes (the host transposes each x shard
on the way in and the output shard on the way out).  LayerNorm reductions
over the channel (partition) dim are done on the TensorEngine as
ones-matmuls whose [128, T] PSUM output broadcasts the per-token sums to
every partition.  Matmul dtypes: fp32r (TF32-like, 1 cycle/row) for the
residual-path stats, fp16 for the three big matmuls (weights cast on host).
"""

from contextlib import ExitStack

import numpy as np

import concourse.bacc as bacc
import concourse.mybir as mybir
import concourse.tile as tile
from concourse.bass_utils import run_bass_kernel_spmd

AF = mybir.ActivationFunctionType
ALU = mybir.AluOpType

P = 128          # SBUF partitions
C = 1024         # channel dim
H = 4096         # MLP hidden dim
KO = C // P      # 8 channel chunks
HO = H // P      # 32 hidden chunks
TOK = 1024       # tokens per core
TT = 512         # token tile (matmul moving dim)
NT = TOK // TT   # 2 token tiles per core
N_CORES = 8
EPS = 1e-5

F32 = mybir.dt.float32
F32R = mybir.dt.float32r
F16 = mybir.dt.float16

# packed param columns (each [1024] vector becomes [128, 8] partition-major)
_PCOLS = {
    "ln1_g": 0, "ln1_b": 8, "lnf_g": 16, "lnf_b": 24,
    "ln2_g": 32, "ln2_b": 40, "b2": 48,
}
_B1_COL = 56  # b1 occupies cols 56..88
_PWIDTH = 88


def _build_nc():
    nc = bacc.Bacc()

    xT = nc.declare_dram_parameter("xT", [C, TOK], F32R, isOutput=False)
    fcos = nc.declare_dram_parameter("fcos", [C, C], F16, isOutput=False)
    w1 = nc.declare_dram_parameter("w1", [C, H], F16, isOutput=False)
    w2 = nc.declare_dram_parameter("w2", [H, C], F16, isOutput=False)
    params = nc.declare_dram_parameter("params", [P, _PWIDTH], F32, isOutput=False)
    outT = nc.declare_dram_parameter("outT", [C, TOK], F32R, isOutput=True)

    xT_r = xT.rearrange("(ko kp) t -> kp ko t", kp=P)
    fcos_r = fcos.rearrange("(ko kp) m -> kp ko m", kp=P)
    w1_r = w1.rearrange("(ko kp) h -> kp ko h", kp=P)
    w2_r = w2.rearrange("(ho hp) c -> hp ho c", hp=P)
    outT_r = outT.rearrange("(co cp) t -> cp co t", cp=P)

    with tile.TileContext(nc) as tc, ExitStack() as ctx:
        persist = ctx.enter_context(tc.tile_pool(name="persist", bufs=1))
        tmp = ctx.enter_context(tc.tile_pool(name="tmp", bufs=3))
        stat = ctx.enter_context(tc.tile_pool(name="stat", bufs=2))
        outp = ctx.enter_context(tc.tile_pool(name="outp", bufs=3))
        ps_s = ctx.enter_context(tc.tile_pool(name="ps_s", bufs=1, space="PSUM"))
        ps_q = ctx.enter_context(tc.tile_pool(name="ps_q", bufs=1, space="PSUM"))
        ps_fft = ctx.enter_context(tc.tile_pool(name="ps_fft", bufs=2, space="PSUM"))
        ps_mlp = ctx.enter_context(tc.tile_pool(name="ps_mlp", bufs=2, space="PSUM"))
        ps_out = ctx.enter_context(tc.tile_pool(name="ps_out", bufs=2, space="PSUM"))

        # ---------- constants ----------
        ones_f = persist.tile([P, P], F32)
        nc.vector.memset(ones_f, 1.0)
        ones_r = persist.tile([P, P], F32R)
        nc.scalar.activation(ones_r, ones_f, AF.Copy)
        ones_h = persist.tile([P, P], F16)
        nc.vector.memset(ones_h, 1.0)
        eps_sb = persist.tile([P, 1], F32)
        nc.vector.memset(eps_sb, EPS)

        par_sb = persist.tile([P, _PWIDTH], F32)
        nc.sync.dma_start(par_sb, params[:, :])

        def pcol(name, k):
            c0 = _PCOLS[name] + k
            return par_sb[:, c0 : c0 + 1]

        # activations that live across both phases
        x2_sb = [persist.tile([P, KO, TT], F32R, name=f"x2{t}") for t in range(NT)]
        h2_sb = [persist.tile([P, KO, TT], F16, name=f"h2{t}") for t in range(NT)]

        def ln_stats(src, ones):
            """src: [P, KO, TT] tile (f32r or f16). Returns (mu, rstd) [P, TT]
            f32, broadcast across all partitions."""
            psum_s = ps_s.tile([P, TT], F32, tag="ps_s")
            psum_q = ps_q.tile([P, TT], F32, tag="ps_q")
            for k in range(KO):
                nc.tensor.matmul(
                    psum_s, lhsT=ones, rhs=src[:, k, :],
                    start=(k == 0), stop=(k == KO - 1),
                )
            for k in range(KO):
                sq = tmp.tile([P, TT], src.dtype, tag="sq")
                nc.vector.tensor_tensor(sq, src[:, k, :], src[:, k, :], ALU.mult)
                nc.tensor.matmul(
                    psum_q, lhsT=ones, rhs=sq,
                    start=(k == 0), stop=(k == KO - 1),
                )
            mu = stat.tile([P, TT], F32, tag="mu")
            nc.scalar.activation(mu, psum_s, AF.Copy, scale=1.0 / C)
            ex2 = stat.tile([P, TT], F32, tag="ex2")
            nc.scalar.activation(ex2, psum_q, AF.Copy, scale=1.0 / C)
            var = stat.tile([P, TT], F32, tag="var")
            nc.vector.tensor_tensor(var, mu, mu, ALU.mult)
            nc.vector.tensor_tensor(var, ex2, var, ALU.subtract)
            nc.scalar.activation(var, var, AF.Sqrt, bias=eps_sb)
            rstd = stat.tile([P, TT], F32, tag="rstd")
            nc.vector.reciprocal(rstd, var)
            return mu, rstd

        def ln_apply(src, mu, rstd, gname, bname, dst):
            """dst[:, k, :] = (src[:, k, :] - mu) * rstd * g[k] + b[k]"""
            for k in range(KO):
                xc = tmp.tile([P, TT], F32, tag="xc")
                nc.vector.tensor_tensor(xc, src[:, k, :], mu, ALU.subtract)
                nc.vector.tensor_tensor(xc, xc, rstd, ALU.mult)
                nc.scalar.activation(
                    dst[:, k, :], xc, AF.Identity,
                    bias=pcol(bname, k), scale=pcol(gname, k),
                )

        # ================= phase 1: LN1 -> FFT -> LNf -> +x -> LN2 =========
        with tc.tile_pool(name="ph1", bufs=1) as ph1:
            fcos_sb = ph1.tile([P, KO, C], F16)
            for k in range(KO):
                nc.sync.dma_start(fcos_sb[:, k, :], fcos_r[:, k, :])

            x_sb = [ph1.tile([P, KO, TT], F32R, name=f"x{t}") for t in range(NT)]
            h_sb = [ph1.tile([P, KO, TT], F16, name=f"h{t}") for t in range(NT)]
            f_sb = [ph1.tile([P, KO, TT], F16, name=f"f{t}") for t in range(NT)]
            for t in range(NT):
                for k in range(KO):
                    nc.sync.dma_start(
                        x_sb[t][:, k, :], xT_r[:, k, t * TT : (t + 1) * TT]
                    )

            for t in range(NT):
                mu1, rstd1 = ln_stats(x_sb[t], ones_r)
                ln_apply(x_sb[t], mu1, rstd1, "ln1_g", "ln1_b", h_sb[t])

                # f = h @ COS (channel-major: psum[c_out, t] = COS[:,co].T @ h)
                for m in range(KO):
                    psum_f = ps_fft.tile([P, TT], F32, tag="fft")
                    for k in range(KO):
                        nc.tensor.matmul(
                            psum_f,
                            lhsT=fcos_sb[:, k, m * P : (m + 1) * P],
                            rhs=h_sb[t][:, k, :],
                            start=(k == 0), stop=(k == KO - 1),
                        )
                    nc.scalar.activation(f_sb[t][:, m, :], psum_f, AF.Copy)

                muf, rstdf = ln_stats(f_sb[t], ones_h)
                # x2 = x + LNf(f)
                for k in range(KO):
                    fn = tmp.tile([P, TT], F32, tag="fn")
                    nc.vector.tensor_tensor(fn, f_sb[t][:, k, :], muf, ALU.subtract)
                    nc.vector.tensor_tensor(fn, fn, rstdf, ALU.mult)
                    nc.scalar.activation(
                        fn, fn, AF.Identity,
                        bias=pcol("lnf_b", k), scale=pcol("lnf_g", k),
                    )
                    nc.vector.tensor_tensor(
                        x2_sb[t][:, k, :], x_sb[t][:, k, :], fn, ALU.add
                    )

                mu2, rstd2 = ln_stats(x2_sb[t], ones_r)
                ln_apply(x2_sb[t], mu2, rstd2, "ln2_g", "ln2_b", h2_sb[t])

        # ================= phase 2: MLP =====================================
        with tc.tile_pool(name="ph2", bufs=1) as ph2:
            m_sb = [ph2.tile([P, HO, TT], F16, name=f"m{t}") for t in range(NT)]

            # MLP1: m = gelu(h2 @ w1 + b1), w1 streamed by hidden chunk
            for h in range(HO):
                w1blk = ph2.tile([P, KO, P], F16, tag="w1blk", bufs=3)
                nc.sync.dma_start(w1blk, w1_r[:, :, h * P : (h + 1) * P])
                for t in range(NT):
                    psum_m = ps_mlp.tile([P, TT], F32, tag="mlp1")
                    for k in range(KO):
                        nc.tensor.matmul(
                            psum_m, lhsT=w1blk[:, k, :], rhs=h2_sb[t][:, k, :],
                            start=(k == 0), stop=(k == KO - 1),
                        )
                    nc.scalar.activation(
                        m_sb[t][:, h, :], psum_m, AF.Gelu,
                        bias=par_sb[:, _B1_COL + h : _B1_COL + h + 1],
                    )

            # MLP2: out = x2 + m @ w2 + b2, w2 streamed by output-channel block
            for c in range(KO):
                w2blk = ph2.tile([P, HO, P], F16, tag="w2blk", bufs=3)
                nc.sync.dma_start(w2blk, w2_r[:, :, c * P : (c + 1) * P])
                for t in range(NT):
                    psum_o = ps_out.tile([P, TT], F32, tag="out")
                    for h in range(HO):
                        nc.tensor.matmul(
                            psum_o,
                            lhsT=w2blk[:, h, :],
                            rhs=m_sb[t][:, h, :],
                            start=(h == 0), stop=(h == HO - 1),
                        )
                    ob = outp.tile([P, TT], F32R, tag="ob")
                    nc.scalar.activation(ob, psum_o, AF.Identity, bias=pcol("b2", c))
                    nc.vector.tensor_tensor(ob, ob, x2_sb[t][:, c, :], ALU.add)
                    nc.sync.dma_start(outT_r[:, c, t * TT : (t + 1) * TT], ob)

    nc.compile()
    return nc


_NC_CACHE: list = []


def _get_nc():
    if not _NC_CACHE:
        _NC_CACHE.append(_build_nc())
    return _NC_CACHE[0]


def _pack_params(inputs):
    p = np.zeros((P, _PWIDTH), np.float32)
    for name, col in _PCOLS.items():
        p[:, col : col + 8] = np.asarray(inputs[name], np.float32).reshape(8, P).T
    p[:, _B1_COL : _B1_COL + HO] = (
        np.asarray(inputs["b1"], np.float32).reshape(HO, P).T
    )
    return p


def _run(inputs, trace=False):
    x = np.asarray(inputs["x"], np.float32)
    B, N, Cc = x.shape
    assert (B * N, Cc) == (N_CORES * TOK, C)
    x2d = x.reshape(B * N, C)

    n = np.arange(C, dtype=np.float64)
    fcos = np.cos((np.outer(n, n) % C) * (2.0 * np.pi / C)).astype(np.float16)

    w1 = np.asarray(inputs["w1"], np.float32).astype(np.float16)
    w2 = np.asarray(inputs["w2"], np.float32).astype(np.float16)
    params = _pack_params(inputs)

    in_maps = []
    for i in range(N_CORES):
        shard = x2d[i * TOK : (i + 1) * TOK, :]
        in_maps.append(
            {
                "xT": np.ascontiguousarray(shard.T),
                "fcos": fcos,
                "w1": w1,
                "w2": w2,
                "params": params,
            }
        )

    nc = _get_nc()
    res = run_bass_kernel_spmd(nc, in_maps, core_ids=list(range(N_CORES)), trace=trace)

    out2d = np.empty((B * N, C), np.float32)
    for i in range(N_CORES):
        out2d[i * TOK : (i + 1) * TOK, :] = res.results[i]["outT"].T
    return out2d.reshape(B, N, C), res


def kernel(**inputs) -> np.ndarray:
    return _run(inputs)[0]


# revision 9
# speedup vs baseline: 1.1069x; 1.1069x over previous
"""Fused Fourier-block kernel for TRN2 (8 NeuronCores, data-parallel).

Reference computation (per token, C=1024, H=4096):
    h  = LN1(x)
    f  = real(FFT_C(h)) = h @ COS            (COS[n,k] = cos(2*pi*n*k/C))
    x2 = x + LNf(f)
    h2 = LN2(x2)
    m  = gelu_exact(h2 @ w1 + b1)
    out = x2 + m @ w2 + b2

Strategy: shard the 4*2048 = 8192 tokens over 8 cores (1024 tokens each).
All device math is done with activations CHANNEL-MAJOR ([channel, token]),
so every matmul consumes weights in their natural [in, out] layout and
chains without any device-side transposes (the host transposes each x shard
on the way in and the output shard on the way out).  LayerNorm reductions
over the channel (partition) dim are done on the TensorEngine as
ones-matmuls whose [128, T] PSUM output broadcasts the per-token sums to
every partition.  Matmul dtypes: fp32r (TF32-like, 1 cycle/row) for the
residual-path stats, fp16 for the three big matmuls (weights cast on host).
"""

from contextlib import ExitStack

import numpy as np

import concourse.bacc as bacc
import concourse.mybir as mybir
import concourse.tile as tile
from concourse.bass_utils import run_bass_kernel_spmd

AF = mybir.ActivationFunctionType
ALU = mybir.AluOpType

P = 128          # SBUF partitions
C = 1024         # channel dim
H = 4096         # MLP hidden dim
KO = C // P      # 8 channel chunks
HO = H // P      # 32 hidden chunks
TOK = 1024       # tokens per core
TT = 512         # token tile (matmul moving dim)
NT = TOK // TT   # 2 token tiles per core
N_CORES = 8
EPS = 1e-5

F32 = mybir.dt.float32
F32R = mybir.dt.float32r
F16 = mybir.dt.float16

# packed param columns (each [1024] vector becomes [128, 8] partition-major)
_PCOLS = {
    "ln1_g": 0, "ln1_b": 8, "lnf_g": 16, "lnf_b": 24,
    "ln2_g": 32, "ln2_b": 40, "b2": 48,
}
_B1_COL = 56  # b1 occupies cols 56..88
_PWIDTH = 88


def _build_nc():
    nc = bacc.Bacc()

    xT = nc.declare_dram_parameter("xT", [C, TOK], F32R, isOutput=False)
    fcos = nc.declare_dram_parameter("fcos", [C, C], F16, isOutput=False)
    w1 = nc.declare_dram_parameter("w1", [C, H], F16, isOutput=False)
    w2 = nc.declare_dram_parameter("w2", [H, C], F16, isOutput=False)
    params = nc.declare_dram_parameter("params", [P, _PWIDTH], F32, isOutput=False)
    outT = nc.declare_dram_parameter("outT", [C, TOK], F32R, isOutput=True)

    xT_r = xT.rearrange("(ko kp) t -> kp ko t", kp=P)
    fcos_r = fcos.rearrange("(ko kp) m -> kp ko m", kp=P)
    w1_r = w1.rearrange("(ko kp) h -> kp ko h", kp=P)
    w2_r = w2.rearrange("(ho hp) c -> hp ho c", hp=P)
    outT_r = outT.rearrange("(co cp) t -> cp co t", cp=P)

    with tile.TileContext(nc) as tc, ExitStack() as ctx:
        persist = ctx.enter_context(tc.tile_pool(name="persist", bufs=1))
        tmp = ctx.enter_context(tc.tile_pool(name="tmp", bufs=3))
        stat = ctx.enter_context(tc.tile_pool(name="stat", bufs=3))
        outp = ctx.enter_context(tc.tile_pool(name="outp", bufs=3))

        # ---------- constants ----------
        ones_f = persist.tile([P, P], F32)
        nc.vector.memset(ones_f, 1.0)
        ones_r = persist.tile([P, P], F32R)
        nc.scalar.activation(ones_r, ones_f, AF.Copy)
        ones_h = persist.tile([P, P], F16)
        nc.vector.memset(ones_h, 1.0)
        eps_sb = persist.tile([P, 1], F32)
        nc.vector.memset(eps_sb, EPS)

        par_sb = persist.tile([P, _PWIDTH], F32)
        nc.sync.dma_start(par_sb, params[:, :])

        def pcol(name, k):
            c0 = _PCOLS[name] + k
            return par_sb[:, c0 : c0 + 1]

        # activations that live across both phases
        x2_sb = [persist.tile([P, KO, TT], F32R, name=f"x2{t}") for t in range(NT)]
        h2_sb = [persist.tile([P, KO, TT], F16, name=f"h2{t}") for t in range(NT)]

        def ln_stats(src, ones, ps_s, ps_q):
            """src: [P, KO, TT] tile (f32r or f16). Returns (mu, rstd) [P, TT]
            f32, broadcast across all partitions."""
            psum_s = ps_s.tile([P, TT], F32, tag="ps_s")
            psum_q = ps_q.tile([P, TT], F32, tag="ps_q")
            for k in range(KO):
                nc.tensor.matmul(
                    psum_s, lhsT=ones, rhs=src[:, k, :],
                    start=(k == 0), stop=(k == KO - 1),
                )
            for k in range(KO):
                sq = tmp.tile([P, TT], src.dtype, tag="sq")
                nc.vector.tensor_tensor(sq, src[:, k, :], src[:, k, :], ALU.mult)
                nc.tensor.matmul(
                    psum_q, lhsT=ones, rhs=sq,
                    start=(k == 0), stop=(k == KO - 1),
                )
            mu = stat.tile([P, TT], F32, tag="mu")
            nc.scalar.activation(mu, psum_s, AF.Copy, scale=1.0 / C)
            ex2 = stat.tile([P, TT], F32, tag="ex2")
            nc.scalar.activation(ex2, psum_q, AF.Copy, scale=1.0 / C)
            var = stat.tile([P, TT], F32, tag="var")
            nc.vector.tensor_tensor(var, mu, mu, ALU.mult)
            nc.vector.tensor_tensor(var, ex2, var, ALU.subtract)
            nc.scalar.activation(var, var, AF.Sqrt, bias=eps_sb)
            rstd = stat.tile([P, TT], F32, tag="rstd")
            nc.vector.reciprocal_approx_fast(rstd, var)
            return mu, rstd

        def ln_apply(src, mu, rstd, gname, bname, dst):
            """dst[:, k, :] = (src[:, k, :] - mu) * rstd * g[k] + b[k]"""
            for k in range(KO):
                xc = tmp.tile([P, TT], F32, tag="xc")
                nc.vector.tensor_tensor(xc, src[:, k, :], mu, ALU.subtract)
                nc.vector.tensor_tensor(xc, xc, rstd, ALU.mult)
                nc.scalar.activation(
                    dst[:, k, :], xc, AF.Identity,
                    bias=pcol(bname, k), scale=pcol(gname, k),
                )

        # ================= phase 1: LN1 -> FFT -> LNf -> +x -> LN2 =========
        # t0/t1 interleaved so PE fills one tile's LN-chain latency with the
        # other tile's matmuls. FFT runs k-outer in halves of 4 PSUM banks so
        # it starts as soon as the first h chunk is ready.
        with tc.tile_pool(name="ph1", bufs=1) as ph1, \
             tc.tile_pool(name="ps_s", bufs=2, space="PSUM") as ps_s, \
             tc.tile_pool(name="ps_q", bufs=2, space="PSUM") as ps_q, \
             tc.tile_pool(name="ps_fft", bufs=4, space="PSUM") as ps_fft:
            x_sb = [ph1.tile([P, KO, TT], F32R, name=f"x{t}") for t in range(NT)]
            h_sb = [ph1.tile([P, KO, TT], F16, name=f"h{t}") for t in range(NT)]
            f_sb = [ph1.tile([P, KO, TT], F16, name=f"f{t}") for t in range(NT)]
            fcos_sb = ph1.tile([P, KO, C], F16)

            for t in range(NT):
                for k in range(KO):
                    nc.sync.dma_start(
                        x_sb[t][:, k, :], xT_r[:, k, t * TT : (t + 1) * TT]
                    )
            for k in range(KO):
                nc.sync.dma_start(fcos_sb[:, k, :], fcos_r[:, k, :])

            st1 = [ln_stats(x_sb[t], ones_r, ps_s, ps_q) for t in range(NT)]
            for t in range(NT):
                ln_apply(x_sb[t], *st1[t], "ln1_g", "ln1_b", h_sb[t])

            # f = h @ COS, k-outer over halves of 4 output chunks
            for t in range(NT):
                for half in range(2):
                    psums = [
                        ps_fft.tile([P, TT], F32, tag="fft", name=f"fft{j}")
                        for j in range(4)
                    ]
                    for k in range(KO):
                        for j in range(4):
                            m = half * 4 + j
                            nc.tensor.matmul(
                                psums[j],
                                lhsT=fcos_sb[:, k, m * P : (m + 1) * P],
                                rhs=h_sb[t][:, k, :],
                                start=(k == 0), stop=(k == KO - 1),
                            )
                    for j in range(4):
                        nc.scalar.activation(
                            f_sb[t][:, half * 4 + j, :], psums[j], AF.Copy
                        )

            stf = [ln_stats(f_sb[t], ones_h, ps_s, ps_q) for t in range(NT)]
            for t in range(NT):
                muf, rstdf = stf[t]
                for k in range(KO):
                    fn = tmp.tile([P, TT], F32, tag="fn")
                    nc.vector.tensor_tensor(fn, f_sb[t][:, k, :], muf, ALU.subtract)
                    nc.vector.tensor_tensor(fn, fn, rstdf, ALU.mult)
                    nc.scalar.activation(
                        fn, fn, AF.Identity,
                        bias=pcol("lnf_b", k), scale=pcol("lnf_g", k),
                    )
                    nc.vector.tensor_tensor(
                        x2_sb[t][:, k, :], x_sb[t][:, k, :], fn, ALU.add
                    )

            st2 = [ln_stats(x2_sb[t], ones_r, ps_s, ps_q) for t in range(NT)]
            for t in range(NT):
                ln_apply(x2_sb[t], *st2[t], "ln2_g", "ln2_b", h2_sb[t])

        # ================= phase 2: MLP =====================================
        with tc.tile_pool(name="ph2", bufs=1) as ph2, \
             tc.tile_pool(name="ps_mlp", bufs=4, space="PSUM") as ps_mlp, \
             tc.tile_pool(name="ps_out", bufs=4, space="PSUM") as ps_out:
            m_sb = [ph2.tile([P, HO, TT], F16, name=f"m{t}") for t in range(NT)]

            # MLP1: m = gelu(h2 @ w1 + b1), w1 streamed by hidden chunk
            for h in range(HO):
                w1blk = ph2.tile([P, KO, P], F16, tag="w1blk", bufs=3)
                nc.sync.dma_start(w1blk, w1_r[:, :, h * P : (h + 1) * P])
                for t in range(NT):
                    psum_m = ps_mlp.tile([P, TT], F32, tag="mlp1")
                    for k in range(KO):
                        nc.tensor.matmul(
                            psum_m, lhsT=w1blk[:, k, :], rhs=h2_sb[t][:, k, :],
                            start=(k == 0), stop=(k == KO - 1),
                        )
                    nc.scalar.activation(
                        m_sb[t][:, h, :], psum_m, AF.Gelu,
                        bias=par_sb[:, _B1_COL + h : _B1_COL + h + 1],
                    )

            # MLP2: out = x2 + m @ w2 + b2, w2 streamed by out-channel block
            for c in range(KO):
                w2blk = ph2.tile([P, HO, P], F16, tag="w2blk", bufs=3)
                nc.sync.dma_start(w2blk, w2_r[:, :, c * P : (c + 1) * P])
                for t in range(NT):
                    psum_o = ps_out.tile([P, TT], F32, tag="out")
                    for h in range(HO):
                        nc.tensor.matmul(
                            psum_o,
                            lhsT=w2blk[:, h, :],
                            rhs=m_sb[t][:, h, :],
                            start=(h == 0), stop=(h == HO - 1),
                        )
                    ob = outp.tile([P, TT], F32R, tag="ob")
                    nc.scalar.activation(ob, psum_o, AF.Identity, bias=pcol("b2", c))
                    nc.vector.tensor_tensor(ob, ob, x2_sb[t][:, c, :], ALU.add)
                    nc.sync.dma_start(outT_r[:, c, t * TT : (t + 1) * TT], ob)

    nc.compile()
    return nc


_NC_CACHE: list = []


def _get_nc():
    if not _NC_CACHE:
        _NC_CACHE.append(_build_nc())
    return _NC_CACHE[0]


def _pack_params(inputs):
    p = np.zeros((P, _PWIDTH), np.float32)
    for name, col in _PCOLS.items():
        p[:, col : col + 8] = np.asarray(inputs[name], np.float32).reshape(8, P).T
    p[:, _B1_COL : _B1_COL + HO] = (
        np.asarray(inputs["b1"], np.float32).reshape(HO, P).T
    )
    return p


def _run(inputs, trace=False):
    x = np.asarray(inputs["x"], np.float32)
    B, N, Cc = x.shape
    assert (B * N, Cc) == (N_CORES * TOK, C)
    x2d = x.reshape(B * N, C)

    n = np.arange(C, dtype=np.float64)
    fcos = np.cos((np.outer(n, n) % C) * (2.0 * np.pi / C)).astype(np.float16)

    w1 = np.asarray(inputs["w1"], np.float32).astype(np.float16)
    w2 = np.asarray(inputs["w2"], np.float32).astype(np.float16)
    params = _pack_params(inputs)

    in_maps = []
    for i in range(N_CORES):
        shard = x2d[i * TOK : (i + 1) * TOK, :]
        in_maps.append(
            {
                "xT": np.ascontiguousarray(shard.T),
                "fcos": fcos,
                "w1": w1,
                "w2": w2,
                "params": params,
            }
        )

    nc = _get_nc()
    res = run_bass_kernel_spmd(nc, in_maps, core_ids=list(range(N_CORES)), trace=trace)

    out2d = np.empty((B * N, C), np.float32)
    for i in range(N_CORES):
        out2d[i * TOK : (i + 1) * TOK, :] = res.results[i]["outT"].T
    return out2d.reshape(B, N, C), res


def kernel(**inputs) -> np.ndarray:
    return _run(inputs)[0]
